# revision 23
# baseline (speedup 1.0000x reference)
"""Trainium2 8-core Bass kernel for nn_AttentionFlow (GNN message passing).

Strategy (per core c of 8):
  - Edges sharded 50000/core, aligned to the 20-edge vi-segment structure
    (2500 segments/core), so the segment softmax is fully core-local.
  - hc = tanh(hidden_con @ Wc + bc): projection row-sharded (16384 rows/core,
    bf16) then AllGather -> full 131072-row table per core in DRAM.
  - hu = tanh(hidden_uncon @ Wu + bu): same, padded to 65536 rows, 8192/core.
  - Per-rel fused tables ABCD[r] = [ws0+ws1*rel | ws2+ws3*rel | ws4+ws5*rel |
    ws6+ws7*rel] * |out_w| (512B bf16 rows) built on device; the F-layer then is
    x = f0*(f3*A + f4*B) + f1*(f3*C + f4*D) (+fb*|w|), 9-10 DVE passes.
  - logits = sum_d sign(w_d) relu(x_d): host permutes the d axis of all
    parameters so positive-sign dims are contiguous -> two free-dim reduces.
  - Per-edge gathers via gpsimd indirect DMA ([P,1] offsets, 128 rows each;
    the SWDGE ucode honors one offset per partition).
  - Softmax per segment entirely per-partition ([128 seg, 20] tiles).
  - Host sums the 8 per-core trans_att outputs into [4, 50000] (np.add.at).
All indices are preprocessed on the host (pure integer remapping).
"""

import sys

sys.path.insert(0, "/opt/trn_rl_repo")

import numpy as np
import ml_dtypes

from concourse import bass, bacc, mybir
import concourse.tile as tile
from concourse.bass_utils import run_bass_kernel_spmd

BF = ml_dtypes.bfloat16

NCORES = 8
B = 4
E = 400_000
EPC = E // NCORES            # 50000 edges per core
KK = 20                      # edges per vi segment
SEGS = EPC // KK             # 2500 segments per core
P = 128
NT = (SEGS + P - 1) // P     # 20 tiles of 128 segments
SEG_PAD = NT * P             # 2560
EPC_PAD = SEG_PAD * KK       # 51200
NN = 50_000
NREL = 500
NRELP = 512
D = 64
DLG = 256
NMEM = 131_072
HC_SH = NMEM // NCORES       # 16384
HU_PAD = 65_536
HU_SH = HU_PAD // NCORES     # 8192

f32 = mybir.dt.float32
bf16 = mybir.dt.bfloat16
i32 = mybir.dt.int32

EDC = 128  # edata columns per segment-partition (see layout below)
# edata columns: 0:20 e2vi', 20:40 e2vj', 40:60 vj', 60:80 rel, 80:100 ey(f32),
#                100 na(f32), 101 viseg', rest pad


def _remap_blk1024(x):
    """hc/hu tables are written in 1024-row blocks as [p(128), j(8)]:
    dram row blk+8p+j holds logical row blk+128j+p."""
    x = np.asarray(x, np.int64)
    return (x & ~np.int64(1023)) + ((x & 127) << 3) + ((x >> 7) & 7)


def _build_proj():
    """Phase A: sharded hc/hu projections (no collectives)."""
    nc = bacc.Bacc("TRN2", target_bir_lowering=False, debug=False,
                   num_devices=NCORES)
    hcon_sh = nc.declare_dram_parameter("hcon_sh", [HC_SH, D], bf16, isOutput=False)
    hun_sh = nc.declare_dram_parameter("hun_sh", [HU_SH, DLG], bf16, isOutput=False)
    wc_ext = nc.declare_dram_parameter("wc_ext", [D + 1, D], f32, isOutput=False)
    wu_ext = nc.declare_dram_parameter("wu_ext", [DLG + 1, D], f32, isOutput=False)
    hc_out = nc.declare_dram_parameter("hc_sh", [HC_SH * D], bf16, isOutput=True)
    hu_out = nc.declare_dram_parameter("hu_sh", [HU_SH * D], bf16, isOutput=True)

    with tile.TileContext(nc) as tc:
        with (
            tc.tile_pool(name="const", bufs=1) as cpool,
            tc.tile_pool(name="proj", bufs=2) as ppool,
            tc.tile_pool(name="psum", bufs=2, space="PSUM") as pspool,
        ):
            ones = cpool.tile([1, P], bf16)
            nc.vector.memset(ones[:], 1.0)
            wc_sb = cpool.tile([D + 1, D], bf16)
            nc.gpsimd.dma_start(out=wc_sb[:], in_=wc_ext[:])
            wu_sb = cpool.tile([DLG // 2, 2, D], bf16)
            nc.gpsimd.dma_start(out=wu_sb[:, 0, :], in_=wu_ext[0:128, :])
            nc.gpsimd.dma_start(out=wu_sb[:, 1, :], in_=wu_ext[128:256, :])
            bu_sb = cpool.tile([1, D], bf16)
            nc.gpsimd.dma_start(out=bu_sb[:], in_=wu_ext[256:257, :])

            CH = 4096
            for ch in range(HC_SH // CH):
                xt = ppool.tile([D + 1, CH], bf16, tag="xt_hc")
                nc.sync.dma_start_transpose(
                    out=xt[0:D, :], in_=hcon_sh[ch * CH:(ch + 1) * CH, :])
                nc.vector.memset(xt[D:D + 1, :], 1.0)
                for b2 in range(CH // 1024):
                    blk = ch * (CH // 1024) + b2
                    ps = pspool.tile([P, 512], f32, space="PSUM", tag="proj_ps")
                    for j in range(8):
                        o = b2 * 1024 + j * 128
                        nc.tensor.matmul(out=ps[:, j * D:(j + 1) * D],
                                         lhsT=xt[:, o:o + 128], rhs=wc_sb[:],
                                         start=True, stop=True)
                    ot = ppool.tile([P, 512], bf16, tag="proj_out")
                    nc.scalar.activation(out=ot[:], in_=ps[:],
                                         func=mybir.ActivationFunctionType.Tanh)
                    nc.sync.dma_start(
                        out=hc_out[blk * 1024 * D:(blk + 1) * 1024 * D],
                        in_=ot[:])

            CHU = 2048
            for ch in range(HU_SH // CHU):
                xu0 = ppool.tile([P, CHU], bf16, tag="xu0")
                xu1 = ppool.tile([P, CHU], bf16, tag="xu1")
                r0 = ch * CHU
                nc.sync.dma_start_transpose(
                    out=xu0[:], in_=hun_sh[r0:r0 + CHU, 0:128])
                nc.sync.dma_start_transpose(
                    out=xu1[:], in_=hun_sh[r0:r0 + CHU, 128:256])
                for b2 in range(CHU // 1024):
                    blk = ch * (CHU // 1024) + b2
                    ps = pspool.tile([P, 512], f32, space="PSUM", tag="proj_ps")
                    for j in range(8):
                        o = b2 * 1024 + j * 128
                        nc.tensor.matmul(out=ps[:, j * D:(j + 1) * D],
                                         lhsT=xu0[:, o:o + 128],
                                         rhs=wu_sb[:, 0, :],
                                         start=True, stop=False)
                        nc.tensor.matmul(out=ps[:, j * D:(j + 1) * D],
                                         lhsT=xu1[:, o:o + 128],
                                         rhs=wu_sb[:, 1, :],
                                         start=False, stop=False)
                        nc.tensor.matmul(out=ps[:, j * D:(j + 1) * D],
                                         lhsT=ones[0:1, :], rhs=bu_sb[:],
                                         start=False, stop=True)
                    ot = ppool.tile([P, 512], bf16, tag="proj_out")
                    nc.scalar.activation(out=ot[:], in_=ps[:],
                                         func=mybir.ActivationFunctionType.Tanh)
                    nc.sync.dma_start(
                        out=hu_out[blk * 1024 * D:(blk + 1) * 1024 * D],
                        in_=ot[:])
    nc.finalize()
    return nc


def _build_main(dp, ellw, add_fb):
    """Phase B: gathers + F-layer + softmax + aggregation (no collectives).
    Returns the per-core partial output; host sums the 8 partials."""
    nc = bacc.Bacc("TRN2", target_bir_lowering=False, debug=False,
                   num_devices=NCORES)
    hc_full = nc.declare_dram_parameter("hc_full", [NMEM, D], bf16, isOutput=False)
    hu_full = nc.declare_dram_parameter("hu_full", [HU_PAD, D], bf16, isOutput=False)
    relt = nc.declare_dram_parameter("relt", [NRELP, D], f32, isOutput=False)
    ws_p = nc.declare_dram_parameter("ws_p", [8, D], f32, isOutput=False)
    outw_p = nc.declare_dram_parameter("outw_p", [1, D], f32, isOutput=False)
    fb_p = nc.declare_dram_parameter("fb_p", [1, D], f32, isOutput=False)
    edata = nc.declare_dram_parameter("edata", [NT, P, EDC], i32, isOutput=False)
    f16 = mybir.dt.float16
    iotab = nc.declare_dram_parameter("iotab", [P, 4, P], f16, isOutput=False)
    relB = nc.declare_dram_parameter("relB", [NT, P, SEG_PAD], f16,
                                     isOutput=False)
    ta_ext = nc.declare_dram_parameter("ta", [P, NT * KK], f32, isOutput=True)

    with tile.TileContext(nc) as tc:
        with (
            tc.tile_pool(name="const", bufs=1) as cpool,
            tc.tile_pool(name="psum", bufs=2, space="PSUM") as pspool,
            tc.tile_pool(name="gat", bufs=2) as gpool,
            tc.tile_pool(name="mid", bufs=2) as mpool,
            tc.tile_pool(name="sm", bufs=3) as spool,
        ):
            ones32 = cpool.tile([1, P], f32)
            nc.vector.memset(ones32[:], 1.0)

            # |out_w|, ws*|w| broadcast tiles, ABCD table
            outw_sb = cpool.tile([1, D], f32)
            nc.sync.dma_start(out=outw_sb[:], in_=outw_p[:])
            absw = cpool.tile([1, D], f32)
            nc.scalar.activation(out=absw[:], in_=outw_sb[:],
                                 func=mybir.ActivationFunctionType.Abs)
            ws_flat = cpool.tile([1, 8 * D], f32)
            nc.sync.dma_start(out=ws_flat[:], in_=ws_p[:])
            psk = pspool.tile([P, 8 * D], f32, space="PSUM", tag="bc_ps")
            nc.tensor.matmul(out=psk[:], lhsT=ones32[0:1, :],
                             rhs=ws_flat[0:1, :], start=True, stop=True)
            psa = pspool.tile([P, D], f32, space="PSUM", tag="bc_ps2")
            nc.tensor.matmul(out=psa[:], lhsT=ones32[0:1, :],
                             rhs=absw[0:1, :], start=True, stop=True)
            abswb = cpool.tile([P, 1, D], f32)
            nc.vector.tensor_copy(out=abswb[:], in_=psa[:])
            wsb_all = cpool.tile([P, 8, D], bf16)
            nc.vector.tensor_tensor(
                out=wsb_all[:],
                in0=psk[:].rearrange("p (a b) -> p a b", a=8),
                in1=abswb[:].to_broadcast([P, 8, D]),
                op=mybir.AluOpType.mult)
            if add_fb:
                fb_sb = cpool.tile([1, D], f32)
                nc.sync.dma_start(out=fb_sb[:], in_=fb_p[:])
                psf = pspool.tile([P, D], f32, space="PSUM", tag="bc_ps2")
                nc.tensor.matmul(out=psf[:], lhsT=ones32[0:1, :],
                                 rhs=fb_sb[0:1, :], start=True, stop=True)
                fbb = cpool.tile([P, 1, D], bf16)
                nc.vector.tensor_tensor(
                    out=fbb[:], in0=psf[:].rearrange("p b -> p 1 b"),
                    in1=abswb[:], op=mybir.AluOpType.mult)

            # abcd_pe[k, w, t, :] = ABCD coeffs of rel row (128w + k), laid
            # out for PE one-hot selection (rel-row on the partition axis).
            rel_sb2 = cpool.tile([P, 4, D], bf16)
            nc.gpsimd.dma_start(
                out=rel_sb2[:],
                in_=relt[:].rearrange("(a b) d -> b a d", a=4))
            abcd_pe = cpool.tile([P, 4, 4, D], bf16)
            for t in range(4):
                tmp = mpool.tile([P, 4, D], bf16, tag="abcd_tmp")
                nc.vector.tensor_tensor(
                    out=tmp[:], in0=rel_sb2[:],
                    in1=wsb_all[:, 2 * t + 1:2 * t + 2, :].to_broadcast([P, 4, D]),
                    op=mybir.AluOpType.mult)
                nc.vector.tensor_tensor(
                    out=abcd_pe[:, :, t, :], in0=tmp[:],
                    in1=wsb_all[:, 2 * t:2 * t + 1, :].to_broadcast([P, 4, D]),
                    op=mybir.AluOpType.add)

            iotab_sb = cpool.tile([P, 4, P], f16)
            nc.sync.dma_start(out=iotab_sb[:], in_=iotab[:])

            ta_all = cpool.tile([P, NT, KK], f32)

            for t in range(NT):
                ed = gpool.tile([P, EDC], i32, tag="ed")
                nc.sync.dma_start(out=ed[:], in_=edata[t])

                def g1(dst, table, idx_ap):
                    nc.gpsimd.indirect_dma_start(
                        out=dst, out_offset=None, in_=table[:],
                        in_offset=bass.IndirectOffsetOnAxis(ap=idx_ap, axis=0))

                f0 = gpool.tile([P, KK, D], bf16, tag="f0")
                f3 = gpool.tile([P, KK, D], bf16, tag="f3")
                f4 = gpool.tile([P, KK, D], bf16, tag="f4")
                g = gpool.tile([P, KK, 4 * D], bf16, tag="g")
                for k in range(KK):
                    g1(f0[:, k, :], hc_full, ed[:, k:k + 1])
                    g1(f3[:, k, :], hc_full, ed[:, 20 + k:21 + k])
                    g1(f4[:, k, :], hu_full, ed[:, 40 + k:41 + k])
                f1 = gpool.tile([P, 1, D], bf16, tag="f1")
                g1(f1[:, 0, :], hu_full, ed[:, 101:102])

                # g[:, j, :] = ABCD[rel of edge (p, j)] via PE one-hot:
                # host-broadcast rel row -> compare against iota -> @ abcd_pe
                rel_bc = gpool.tile([P, KK, P], f16, tag="rel_bc")
                nc.sync.dma_start(out=rel_bc[:], in_=relB[t])
                for j in range(KK):
                    onehot = mpool.tile([P, 4, P], bf16, tag="onehot")
                    nc.vector.tensor_tensor(
                        out=onehot[:],
                        in0=rel_bc[:, j:j + 1, :].to_broadcast([P, 4, P]),
                        in1=iotab_sb[:],
                        op=mybir.AluOpType.is_equal)
                    gps = pspool.tile([P, 4 * D], f32, space="PSUM", tag="gps")
                    for w in range(4):
                        nc.tensor.matmul(
                            out=gps[:],
                            lhsT=onehot[:, w, :],
                            rhs=abcd_pe[:, w, :, :].rearrange("p a b -> p (a b)"),
                            start=(w == 0), stop=(w == 3))
                    nc.scalar.activation(
                        out=g[:, j, :], in_=gps[:],
                        func=mybir.ActivationFunctionType.Copy)

                TT = nc.vector.tensor_tensor
                MU = mybir.AluOpType.mult
                AD = mybir.AluOpType.add
                u1 = mpool.tile([P, KK, D], bf16, tag="u1")
                TT(out=u1[:], in0=f3[:], in1=g[:, :, 0 * D:1 * D], op=MU)
                u2 = mpool.tile([P, KK, D], bf16, tag="u2")
                TT(out=u2[:], in0=f4[:], in1=g[:, :, 1 * D:2 * D], op=MU)
                TT(out=u1[:], in0=u1[:], in1=u2[:], op=AD)
                TT(out=u2[:], in0=u1[:], in1=f0[:], op=MU)
                u3 = mpool.tile([P, KK, D], bf16, tag="u3")
                TT(out=u3[:], in0=f3[:], in1=g[:, :, 2 * D:3 * D], op=MU)
                u4 = mpool.tile([P, KK, D], bf16, tag="u4")
                TT(out=u4[:], in0=f4[:], in1=g[:, :, 3 * D:4 * D], op=MU)
                TT(out=u3[:], in0=u3[:], in1=u4[:], op=AD)
                TT(out=u4[:], in0=u3[:],
                   in1=f1[:].to_broadcast([P, KK, D]), op=MU)
                x = mpool.tile([P, KK, D], f32, tag="x")
                TT(out=x[:], in0=u2[:], in1=u4[:], op=AD)
                if add_fb:
                    TT(out=x[:], in0=x[:],
                       in1=fbb[:].to_broadcast([P, KK, D]), op=AD)
                xr = mpool.tile([P, KK, D], f32, tag="xr")
                nc.scalar.activation(out=xr[:], in_=x[:],
                                     func=mybir.ActivationFunctionType.Relu)

                logit = spool.tile([P, KK], f32, tag="logit")
                if dp == D:
                    nc.vector.tensor_reduce(out=logit[:], in_=xr[:],
                                            axis=mybir.AxisListType.X, op=AD)
                elif dp == 0:
                    neg = spool.tile([P, KK], f32, tag="neg")
                    nc.vector.tensor_reduce(out=neg[:], in_=xr[:],
                                            axis=mybir.AxisListType.X, op=AD)
                    nc.vector.tensor_scalar_mul(logit[:], neg[:], -1.0)
                else:
                    pos = spool.tile([P, KK], f32, tag="pos")
                    nc.vector.tensor_reduce(out=pos[:], in_=xr[:, :, 0:dp],
                                            axis=mybir.AxisListType.X, op=AD)
                    neg = spool.tile([P, KK], f32, tag="neg")
                    nc.vector.tensor_reduce(out=neg[:], in_=xr[:, :, dp:D],
                                            axis=mybir.AxisListType.X, op=AD)
                    nc.vector.scalar_tensor_tensor(
                        out=logit[:], in0=pos[:], scalar=1.0, in1=neg[:],
                        op0=MU, op1=mybir.AluOpType.subtract)

                m = spool.tile([P, 1], f32, tag="m")
                nc.vector.tensor_reduce(out=m[:], in_=logit[:],
                                        axis=mybir.AxisListType.X,
                                        op=mybir.AluOpType.max)
                mneg = spool.tile([P, 1], f32, tag="mneg")
                nc.vector.tensor_scalar_mul(mneg[:], m[:], -1.0)
                ex = spool.tile([P, KK], f32, tag="ex")
                den = spool.tile([P, 1], f32, tag="den")
                nc.scalar.activation(out=ex[:], in_=logit[:],
                                     func=mybir.ActivationFunctionType.Exp,
                                     bias=mneg[:], scale=1.0,
                                     accum_out=den[:])
                rec = spool.tile([P, 1], f32, tag="rec")
                nc.vector.reciprocal(rec[:], den[:])
                sc = spool.tile([P, 1], f32, tag="sc")
                nc.vector.tensor_tensor(out=sc[:], in0=rec[:],
                                        in1=ed[:, 100:101].bitcast(f32), op=MU)
                nc.vector.scalar_tensor_tensor(
                    out=ta_all[:, t, :], in0=ex[:], scalar=sc[:],
                    in1=ed[:, 80:100].bitcast(f32), op0=MU, op1=MU)

            nc.sync.dma_start(
                out=ta_ext[:],
                in_=ta_all[:].rearrange("p a b -> p (a b)"))
    nc.finalize()
    return nc


_CACHE = {}


def _prep(inputs):
    """Host-side: permute the d axis by out_w sign, shard + remap indices."""
    na = np.asarray(inputs["node_attention"], np.float32)
    se = np.asarray(inputs["scanned_edges"])
    ey = np.asarray(inputs["edges_y"], np.float32)
    huncon = np.asarray(inputs["hidden_uncon"], np.float32)[0]
    hcon = np.asarray(inputs["hidden_con"], np.float32)
    Wc = np.asarray(inputs["Wc"], np.float32)
    bc = np.asarray(inputs["bc"], np.float32)
    Wu = np.asarray(inputs["Wu"], np.float32)
    bu = np.asarray(inputs["bu"], np.float32)
    relt = np.asarray(inputs["rel_table"], np.float32)
    ws = np.asarray(inputs["ws"], np.float32)
    fb = np.asarray(inputs["fb"], np.float32)
    out_w = np.asarray(inputs["out_w"], np.float32)

    # d-permutation: positive out_w dims first
    perm = np.argsort(out_w <= 0, kind="stable")
    dp = int((out_w > 0).sum())
    Wcp = np.concatenate([Wc[:, perm], bc[perm][None]], 0)      # [65, 64]
    Wup = np.concatenate([Wu[:, perm], bu[perm][None]], 0)      # [257, 64]
    reltp = np.zeros((NRELP, D), np.float32)
    reltp[:NREL] = relt[:, perm]
    wsp = ws[:, perm].copy()
    outwp = out_w[perm][None].copy()
    fbp = fb[perm][None].copy()
    add_fb = bool(np.any(fb != 0))

    eg, vi, vj, rel, idx_vi, idx_vj, e2vi, e2vj = (
        se[:, i].astype(np.int64) for i in range(8))

    hcon_bf = hcon.astype(BF)
    hun_pad = np.zeros((HU_PAD, DLG), BF)
    hun_pad[:NN] = huncon.astype(BF)

    e2vi_r = _remap_blk1024(e2vi)
    e2vj_r = _remap_blk1024(e2vj)
    vj_r = _remap_blk1024(vj)

    # iotab[p, w, n] = p + 128*w (rel-row id of PE one-hot window w)
    iotab_arr = np.broadcast_to(
        (np.arange(P, dtype=np.float16)[:, None, None]
         + np.float16(128.0) * np.arange(4, dtype=np.float16)[None, :, None]),
        (P, 4, P)).astype(np.float16)

    in_maps = []
    for c in range(NCORES):
        s = c * EPC
        bb = c // 2

        def padseg(a, fill=0):
            out = np.full((SEG_PAD, KK), fill, a.dtype)
            out.reshape(-1)[:EPC] = a
            return out

        ed = np.zeros((NT, P, EDC), np.int32)
        ed[:, :, 0:20] = padseg(e2vi_r[s:s + EPC]).reshape(NT, P, KK)
        ed[:, :, 20:40] = padseg(e2vj_r[s:s + EPC]).reshape(NT, P, KK)
        ed[:, :, 40:60] = padseg(vj_r[s:s + EPC]).reshape(NT, P, KK)
        ed[:, :, 60:80] = padseg(rel[s:s + EPC].astype(np.int32)
                                 ).reshape(NT, P, KK)
        ed[:, :, 80:100] = padseg(ey[s:s + EPC]).reshape(
            NT, P, KK).view(np.int32)
        nav = np.zeros(SEG_PAD, np.float32)
        nav[:SEGS] = na[bb, vi[s:s + EPC][::KK]]
        ed[:, :, 100] = nav.reshape(NT, P).view(np.int32)
        visr = np.zeros(SEG_PAD, np.int64)
        visr[:SEGS] = _remap_blk1024(vi[s:s + EPC][::KK])
        ed[:, :, 101] = visr.reshape(NT, P)

        # relB[t, p, j*128 + n] = rel of edge (n, j) in tile t (host-broadcast
        # down the partition axis, exact in fp16)
        rel_seg = padseg(rel[s:s + EPC].astype(np.int32)).reshape(NT, P, KK)
        relB_arr = np.empty((NT, P, SEG_PAD), np.float16)
        for t in range(NT):
            row = rel_seg[t].T.ravel().astype(np.float16)   # [j*128+n]
            relB_arr[t] = row[None, :]

        in_maps.append({
            "hcon_sh": hcon_bf[c * HC_SH:(c + 1) * HC_SH],
            "hun_sh": hun_pad[c * HU_SH:(c + 1) * HU_SH],
            "wc_ext": Wcp, "wu_ext": Wup, "relt": reltp, "ws_p": wsp,
            "outw_p": outwp, "fb_p": fbp, "edata": ed,
            "iotab": iotab_arr, "relB": relB_arr,
        })
    return in_maps, dp, 64, add_fb, (eg, vj)


def kernel(**inputs):
    in_maps, dp, ellw, add_fb, agg = _prep(inputs)
    if "proj" not in _CACHE:
        _CACHE["proj"] = _build_proj()
    key = ("main", dp, add_fb)
    if key not in _CACHE:
        _CACHE[key] = _build_main(dp, ellw, add_fb)

    proj_keys = ("hcon_sh", "hun_sh", "wc_ext", "wu_ext")
    resA = run_bass_kernel_spmd(
        _CACHE["proj"], [{k: m[k] for k in proj_keys} for m in in_maps],
        core_ids=list(range(NCORES)))
    hc_full = np.concatenate(
        [np.asarray(r["hc_sh"]).reshape(HC_SH, D) for r in resA.results], 0)
    hu_full = np.concatenate(
        [np.asarray(r["hu_sh"]).reshape(HU_SH, D) for r in resA.results], 0)

    main_keys = ("relt", "ws_p", "outw_p", "fb_p", "edata", "iotab", "relB")
    in_maps_b = [{**{k: m[k] for k in main_keys},
                  "hc_full": hc_full, "hu_full": hu_full} for m in in_maps]
    resB = run_bass_kernel_spmd(_CACHE[key], in_maps_b,
                                core_ids=list(range(NCORES)))
    eg_all, vj_all = agg
    out = np.zeros((B, NN), np.float32)
    for c in range(NCORES):
        ta = np.asarray(resB.results[c]["ta"]).reshape(P, NT, KK)
        ta_edges = ta.transpose(1, 0, 2).reshape(-1)[:EPC]
        s = c * EPC
        np.add.at(out, (eg_all[s:s + EPC], vj_all[s:s + EPC]), ta_edges)
    return out
